# revision 23
# baseline (speedup 1.0000x reference)
# Trainium2 Bass kernel for nn_Attention_63900523430102.
#
# Reference computes, for q,k,v of shape (S=2048, B=4, D=1024):
#   xq = q @ wq.T, xk = k @ wk.T, xv = v @ wv.T  (per-head split, hd=64, H=16)
#   xq, xk = rope(xq), rope(xk)
#   scores = softmax(xq . xk / sqrt(hd)) ; out = (scores @ xv) @ wo.T
#
# Sharding: 8 cores = 4 batches x 2 head-groups (8 heads each).  Each core
# gets host-pretransposed operands so that on-device:
#   xqT/xkT [hd, S] come straight out of the projection matmuls,
#   scoresT [l, s] needs no transposes, softmax denominator comes from a
#   ones-column appended to xv (M=65 PV matmul), and the wo partial products
#   are summed pairwise on the host.
#
# Schedule: there is no separate projection phase.  The (head, l-tile)
# score/exp/PV stream for s-block 0 starts as soon as xq(pair0, sb0) and
# xk(pair0, k-block0) exist; all remaining input DMAs, k/v projections and
# rope work are emitted just-in-time in chunked bursts inside the first
# heads' slack, so the Scalar engine (softmax exp, the hard floor at
# ~287us busy) never waits on a bulk phase A.  The score->exp->PV pipeline
# runs continuously across s-block boundaries (a drain there costs ~10us
# per boundary in ACT idle + PE p-state decay).  DMA issuance lives on
# sync+gpsimd only; the scalar queue carries exp exclusively.  k is full-
# resident (two-chunk DMA per tile), v staged through a rotating 8-slice
# pool, and rope tables are bf16 so everything fits SBUF concurrently.
import sys
import os

sys.path.insert(0, "/opt/trn_rl_repo")

import numpy as np
import ml_dtypes

import concourse.bass as bass
import concourse.bacc as bacc
import concourse.mybir as mybir
import concourse.tile as tile
from concourse.bass_utils import run_bass_kernel_spmd


def _shim_axon_hooks():
    """Provide antenv.axon_hooks (NTFF profile hook) if the image lacks it."""
    try:
        from antenv.axon_hooks import get_axon_ntff_profile_hook  # noqa: F401
        return
    except ImportError:
        pass
    import types
    import ctypes
    import contextlib

    so_path = "/opt/axon/libaxon_pjrt.so"
    hook = None
    if os.path.exists(so_path):
        lib = ctypes.CDLL(so_path)
        if hasattr(lib, "axon_start_nrt_profile"):
            lib.axon_start_nrt_profile.argtypes = [
                ctypes.POINTER(ctypes.c_int64), ctypes.c_size_t]
            lib.axon_start_nrt_profile.restype = ctypes.c_int64
            lib.axon_stop_nrt_profile.argtypes = [ctypes.c_char_p]
            lib.axon_stop_nrt_profile.restype = ctypes.c_int64

            @contextlib.contextmanager
            def hook(output_dir, device_ids):
                import jax
                jax.devices()
                if device_ids:
                    ids = (ctypes.c_int64 * len(device_ids))(*device_ids)
                    rc = lib.axon_start_nrt_profile(ids, len(device_ids))
                else:
                    rc = lib.axon_start_nrt_profile(None, 0)
                if rc != 0:
                    raise RuntimeError(f"axon_start_nrt_profile rc={rc}")
                try:
                    yield
                finally:
                    n = lib.axon_stop_nrt_profile(str(output_dir).encode())
                    print(f"ntff profile: {n} file(s) -> {output_dir}",
                          file=sys.stderr)

    mod = types.ModuleType("antenv.axon_hooks")
    mod.get_axon_ntff_profile_hook = lambda: hook
    mod.set_axon_ntff_profile_hook = lambda h: None
    sys.modules["antenv.axon_hooks"] = mod


_shim_axon_hooks()

S = 2048          # seq len (both query s and key l)
D = 1024          # d_model
B = 4             # batch
HLOC = 8          # heads per core
HD = 64           # head dim
E = HLOC * HD     # 512, local e-width per core
NCORES = 8
NPAIR = 4         # head pairs per core (2 heads stacked -> 128 partitions)
NSB = 4           # s blocks of 512
NLT = 16          # l tiles of 128
ND = 8            # d_model tiles of 128
NST = 16          # s tiles of 128 (wo phase)
GLEN = 2          # (head, l-tile) units per exp group

BF16 = mybir.dt.bfloat16
F32 = mybir.dt.float32
NPBF16 = ml_dtypes.bfloat16

_PROG = None
LAST_RESULT = None


def _emit(nc, tc, aps):
    qT, kT, vT, wqT, wkT, wvT, woT, ctab, stab, out = aps
    Exp = mybir.ActivationFunctionType.Exp
    swap_mask = [i ^ 1 for i in range(32)]

    from contextlib import ExitStack

    with ExitStack() as stk:
        consts = stk.enter_context(tc.tile_pool(name="consts", bufs=1))
        persist = stk.enter_context(tc.tile_pool(name="persist", bufs=1))
        qfull = stk.enter_context(tc.tile_pool(name="qfull", bufs=9))
        kfp = stk.enter_context(tc.tile_pool(name="kfull", bufs=1))
        vsl = stk.enter_context(tc.tile_pool(name="vsl", bufs=8))
        rope_pool = stk.enter_context(tc.tile_pool(name="rope", bufs=3))
        probs_pool = stk.enter_context(tc.tile_pool(name="probs", bufs=8))
        small_pool = stk.enter_context(tc.tile_pool(name="small", bufs=2))
        outp = stk.enter_context(tc.tile_pool(name="outp", bufs=3))
        scB = stk.enter_context(tc.tile_pool(name="scB", bufs=3, space="PSUM"))
        pvP = stk.enter_context(tc.tile_pool(name="pvP", bufs=2, space="PSUM"))

        # DMA issuance: sync + gpsimd round-robin; scalar does exp ONLY.
        _dq = [0]

        def q2():
            _dq[0] += 1
            return (nc.sync, nc.gpsimd)[_dq[0] % 2]

        # ---- persistent constants / activations ----
        wq_t = [consts.tile([128, E], BF16, tag=f"wq{d}", name=f"wq{d}")
                for d in range(ND)]
        wk_t = [consts.tile([128, E], BF16, tag=f"wk{d}", name=f"wk{d}")
                for d in range(ND)]
        wv_t = [consts.tile([128, E], BF16, tag=f"wv{d}", name=f"wv{d}")
                for d in range(ND)]
        wo_t = [consts.tile([128, D], BF16, tag=f"wo{e}", name=f"wo_{e}")
                for e in range(NPAIR)]
        ct_t = consts.tile([128, S], BF16, tag="ct", name="ct")
        st_t = consts.tile([128, S], BF16, tag="st", name="st")

        xq_sb = [persist.tile([128, S], BF16, tag=f"xq{p}", name=f"xq{p}")
                 for p in range(NPAIR)]
        xk_sb = [persist.tile([128, S], BF16, tag=f"xk{p}", name=f"xk{p}")
                 for p in range(NPAIR)]
        xv_sb = [persist.tile([128, HLOC * (HD + 1)], BF16,
                              tag=f"xv{lt}", name=f"xv{lt}")
                 for lt in range(NLT)]
        attT = [persist.tile([128, S], BF16, tag=f"att{p}", name=f"att{p}")
                for p in range(NPAIR)]

        # ---- initial DMA burst ----
        # Rope tables first (cheap, unblock the p-state warm-up), then a
        # d-interleaved (wq, wk, q0, k0) burst so the two pre-stream
        # projection chains can start accumulating after the first d-tile
        # lands instead of after the whole 3MB burst.
        nc.sync.dma_start(out=ct_t, in_=ctab[:, :])
        nc.gpsimd.dma_start(out=st_t, in_=stab[:, :])
        qts_of = {}

        def load_q_sb(sb):
            scol = slice(sb * 512, (sb + 1) * 512)
            qts = []
            for d in range(ND):
                t = qfull.tile([128, 512], BF16, tag="qt", name=f"qt{sb}_{d}")
                q2().dma_start(out=t, in_=qT[d * 128:(d + 1) * 128, scol])
                qts.append(t)
            qts_of[sb] = qts

        kf = [kfp.tile([128, S], BF16, tag=f"kf{d}", name=f"kf{d}")
              for d in range(ND)]
        qts = []
        for d in range(ND):
            q2().dma_start(out=wq_t[d], in_=wqT[d * 128:(d + 1) * 128, :])
            q2().dma_start(out=wk_t[d], in_=wkT[d * 128:(d + 1) * 128, :])
            t = qfull.tile([128, 512], BF16, tag="qt", name=f"qt0_{d}")
            q2().dma_start(out=t, in_=qT[d * 128:(d + 1) * 128, 0:512])
            qts.append(t)
            q2().dma_start(out=kf[d][:, 0:512],
                           in_=kT[d * 128:(d + 1) * 128, 0:512])
        qts_of[0] = qts
        for d in range(ND):
            q2().dma_start(out=wv_t[d], in_=wvT[d * 128:(d + 1) * 128, :])

        # v 512-col slices, loaded JIT per sbk and rotated (consumed by
        # v_proj within two hooks of the load).
        def load_v_slices(sbk):
            scol = slice(sbk * 512, (sbk + 1) * 512)
            ts = []
            for d in range(ND):
                t = vsl.tile([128, 512], BF16, tag="vs", name=f"vs{sbk}_{d}")
                q2().dma_start(out=t, in_=vT[d * 128:(d + 1) * 128, scol])
                ts.append(t)
            return ts

        v_slices = {}
        kf_loaded = {0}

        def load_k_chunk(sbk):
            if sbk in kf_loaded:
                return
            kf_loaded.add(sbk)
            scol = slice(sbk * 512, (sbk + 1) * 512)
            for d in range(ND):
                q2().dma_start(out=kf[d][:, scol],
                               in_=kT[d * 128:(d + 1) * 128, scol])

        def rope(ps, dst, sbk):
            cols = slice(sbk * 512, (sbk + 1) * 512)
            t1 = rope_pool.tile([128, 512], F32, tag="t1", name="t1")
            nc.vector.tensor_mul(t1, ps, ct_t[:, cols])
            sw = rope_pool.tile([128, 512], F32, tag="sw", name="sw")
            nc.vector.stream_shuffle(sw, ps, swap_mask)
            t2 = rope_pool.tile([128, 512], F32, tag="t2", name="t2")
            nc.vector.tensor_mul(t2, sw, st_t[:, cols])
            nc.vector.tensor_add(dst[:, cols], t1, t2)

        def xq_proj(p, sb, fold=True):
            # xq projection for (pair p, s-block sb) into a borrowed scB slot
            pcol = slice(p * 128, (p + 1) * 128)
            ps = scB.tile([128, 1024], F32, tag="sc", name="xqps")[:, 0:512]
            for d in range(ND):
                nc.tensor.matmul(ps, lhsT=wq_t[d][:, pcol],
                                 rhs=qts_of[sb][d],
                                 start=(d == 0), stop=(d == ND - 1))
            rope(ps, xq_sb[p], sb)

        def xk_proj(p, sbk):
            pcol = slice(p * 128, (p + 1) * 128)
            scol = slice(sbk * 512, (sbk + 1) * 512)
            ps = scB.tile([128, 1024], F32, tag="sc", name="xkps")[:, 0:512]
            for d in range(ND):
                nc.tensor.matmul(ps, lhsT=wk_t[d][:, pcol],
                                 rhs=kf[d][:, scol],
                                 start=(d == 0), stop=(d == ND - 1))
            rope(ps, xk_sb[p], sbk)

        def v_proj(lt):
            # xv for l-tile lt (all 8 heads) + ones column
            sbk = lt // 4
            lcol = slice((lt % 4) * 128, (lt % 4) * 128 + 128)
            ps = scB.tile([128, 1024], F32, tag="sc", name="xvps")[:, 0:512]
            vs = v_slices[sbk]
            for d in range(ND):
                nc.tensor.matmul(ps, lhsT=vs[d][:, lcol],
                                 rhs=wv_t[d], start=(d == 0), stop=(d == ND - 1))
            dst = xv_sb[lt].rearrange("p (h c) -> p h c", c=HD + 1)
            src = ps.rearrange("p (h c) -> p h c", c=HD)
            nc.vector.tensor_copy(dst[:, :, 0:HD], src)
            nc.vector.memset(dst[:, :, HD], 1.0)

        def normalize(pvt, h, sb):
            p, lh = h // 2, h % 2
            hrow = slice(lh * 64, (lh + 1) * 64)
            scol = slice(sb * 512, (sb + 1) * 512)
            den = small_pool.tile([1, 512], F32, tag="den", name="den")
            nc.vector.tensor_copy(den, pvt[HD:HD + 1, :])
            rc = small_pool.tile([1, 512], F32, tag="rc", name="rc")
            nc.vector.reciprocal_approx_fast(out=rc, in_=den)
            rb = small_pool.tile([64, 512], F32, tag="rb", name="rb")
            nc.gpsimd.partition_broadcast(rb, rc)
            nc.vector.tensor_mul(attT[p][hrow, scol], pvt[0:HD, :], rb)

        def wo_group(sb_prev, h, fold=True):
            st = 4 * sb_prev + h // 2
            nb = h % 2
            trow = slice(st * 128, (st + 1) * 128)
            ps = scB.tile([128, 1024], F32, tag="sc", name="wops")[:, 0:512]
            for et in range(NPAIR):
                nc.tensor.matmul(
                    ps,
                    lhsT=attT[et][:, trow],
                    rhs=wo_t[et][:, nb * 512:(nb + 1) * 512],
                    start=(et == 0), stop=(et == NPAIR - 1))
            ot = outp.tile([128, 512], F32, tag="ot", name="ot")
            nc.vector.tensor_copy(ot, ps)
            q2().dma_start(out=out[trow, nb * 512:(nb + 1) * 512], in_=ot)

        # ---- pre-stream: minimum work to start (h0, sb0) scores ----
        # Warm-up matmuls on the (early-landing) rope table raise the PE out
        # of its cold p-state during the DMA wait; results are never read.
        warm = scB.tile([128, 1024], F32, tag="sc", name="warm")
        for i in range(8):
            nc.tensor.matmul(warm[:, 0:512], lhsT=ct_t[:, 0:128],
                             rhs=ct_t[:, 512:1024], start=True, stop=True)
        # xq(p0, sb0) and xk(p0, block0) accumulate d-interleaved in two
        # PSUM slots, tracking the DMA arrival order.
        ps_q = scB.tile([128, 1024], F32, tag="sc", name="ps_q")
        ps_k = scB.tile([128, 1024], F32, tag="sc", name="ps_k")
        for d in range(ND):
            nc.tensor.matmul(ps_q[:, 0:512], lhsT=wq_t[d][:, 0:128],
                             rhs=qts_of[0][d],
                             start=(d == 0), stop=(d == ND - 1))
            nc.tensor.matmul(ps_k[:, 0:512], lhsT=wk_t[d][:, 0:128],
                             rhs=kf[d][:, 0:512],
                             start=(d == 0), stop=(d == ND - 1))
        rope(ps_q[:, 0:512], xq_sb[0], 0)
        rope(ps_k[:, 0:512], xk_sb[0], 0)

        # Just-in-time emissions executed right before a group's score
        # matmuls.  sb0 absorbs everything the old phase A used to do
        # serially: h0 carries the v-projections (PV trails scores by two
        # groups) plus the k-chunk prefetches, h1..h3 carry one xk+xq pair
        # each, so by h2/h4/h6 the pair it needs is always ready.
        def jit_hook(sb, h, g):
            if sb != 0:
                return
            lt0 = 2 * g
            if h == 0:
                # chunked: 4 v-projections per even hook keep the PE in long
                # bursts (p-state) while the pend-3 exp queue rides the gap.
                if g % 2 == 0:
                    if g == 0:
                        v_slices[0] = load_v_slices(0)
                    sbk = g // 2
                    if sbk >= 1 and sbk not in v_slices:
                        v_slices[sbk] = load_v_slices(sbk)
                    for lt in range(4 * sbk, 4 * sbk + 4):
                        if lt not in vproj_done:
                            v_proj(lt)
                            vproj_done.add(lt)
                sbk = (lt0 + 2) // 4  # one group ahead
                if sbk <= 3 and sbk not in kf_loaded:
                    load_k_chunk(sbk)
                    xk_proj(0, sbk)
            elif h in (1, 2, 3):
                p = h  # pair h, needed first by head 2h
                if g in (0, 2):
                    for sbk in (g, g + 1):
                        if (p, sbk) not in xk_done:
                            xk_proj(p, sbk)
                            xk_done.add((p, sbk))
                elif g == 4:
                    if p not in xq_done:
                        xq_proj(p, 0, fold=False)
                        xq_done.add(p)
            elif h == 6 and g == 0:
                for e in range(NPAIR):
                    q2().dma_start(out=wo_t[e],
                                   in_=woT[e * 128:(e + 1) * 128, :])

        vproj_done = set()
        xk_done = set()
        xq_done = set()

        # ====== flat (s-block, head, l-tile) stream, continuous pend ======
        # No drain at s-block boundaries: the score->exp->PV pipeline runs
        # uninterrupted so neither PE nor ACT sees a bubble (PE pstate decay
        # was costing ~40us when each boundary drained).
        pvt_of = {}

        def emit_scores(sb, g0, glen):
            h, g = g0 // NLT, (g0 % NLT) // GLEN
            jit_hook(sb, h, g)
            scol = slice(sb * 512, (sb + 1) * 512)
            sc = scB.tile([128, 1024], F32, tag="sc", name="sc")
            for j in range(glen):
                u = g0 + j
                h, lt = u // NLT, u % NLT
                p, lh = h // 2, h % 2
                hrow = slice(lh * 64, (lh + 1) * 64)
                nc.tensor.matmul(
                    sc[:, j * 512:(j + 1) * 512],
                    lhsT=xk_sb[p][hrow, lt * 128:(lt + 1) * 128],
                    rhs=xq_sb[p][hrow, scol],
                    start=True, stop=True)
            pr = probs_pool.tile([128, 1024], BF16, tag="pr", name="pr")
            nc.scalar.activation(pr[:, :glen * 512], sc[:, :glen * 512],
                                 Exp, scale=0.125)
            return pr

        def emit_pv(pr, sb, g0, glen):
            for j in range(glen):
                u = g0 + j
                h, lt = u // NLT, u % NLT
                if lt == 0:
                    pvt_of[(sb, h)] = pvP.tile([128, 512], F32,
                                               tag="pv", name="pv")
                nc.tensor.matmul(
                    pvt_of[(sb, h)][0:HD + 1, :],
                    lhsT=xv_sb[lt][:, 65 * h:65 * h + 65],
                    rhs=pr[:, j * 512:(j + 1) * 512],
                    start=(lt == 0), stop=(lt == NLT - 1))
                if lt == NLT - 1:
                    normalize(pvt_of.pop((sb, h)), h, sb)
                    # deferred work folded in at head boundaries:
                    #   h4: q(sb+1) DMA (its last reader, xq(3,sb), ran
                    #       at h2, keeping the 9-deep qfull ring safe)
                    #   h5/h6: xq pairs 0/1 of sb+1 (used from sb+1 h0/h2)
                    #   sb>0 h0/h2: xq pairs 2/3 of sb (used at h4/h6)
                    if sb < NSB - 1:
                        if h == 4:
                            load_q_sb(sb + 1)
                        elif h == 5:
                            xq_proj(0, sb + 1)
                        elif h == 6:
                            xq_proj(1, sb + 1)
                    if sb > 0:
                        if h == 0:
                            xq_proj(2, sb)
                        elif h == 2:
                            xq_proj(3, sb)
                        wo_group(sb - 1, h)

        NU = HLOC * NLT
        pend = []
        for sb in range(NSB):
            for g0 in range(0, NU, GLEN):
                pr = emit_scores(sb, g0, GLEN)
                pend.append((pr, sb, g0, GLEN))
                if len(pend) > 4:
                    emit_pv(*pend.pop(0))
        for args in pend:
            emit_pv(*args)

        # WO tail for the last s-block
        for h in range(HLOC):
            wo_group(NSB - 1, h)


def build_program():
    nc = bacc.Bacc("TRN2", target_bir_lowering=False, debug=False)
    qT = nc.dram_tensor("qT", [D, S], BF16, kind="ExternalInput").ap()
    kT = nc.dram_tensor("kT", [D, S], BF16, kind="ExternalInput").ap()
    vT = nc.dram_tensor("vT", [D, S], BF16, kind="ExternalInput").ap()
    wqT = nc.dram_tensor("wqT", [D, E], BF16, kind="ExternalInput").ap()
    wkT = nc.dram_tensor("wkT", [D, E], BF16, kind="ExternalInput").ap()
    wvT = nc.dram_tensor("wvT", [D, E], BF16, kind="ExternalInput").ap()
    woT = nc.dram_tensor("woT", [E, D], BF16, kind="ExternalInput").ap()
    ctab = nc.dram_tensor("ct", [128, S], BF16, kind="ExternalInput").ap()
    stab = nc.dram_tensor("st", [128, S], BF16, kind="ExternalInput").ap()
    out = nc.dram_tensor("out", [S, D], F32, kind="ExternalOutput").ap()
    aps = (qT, kT, vT, wqT, wkT, wvT, woT, ctab, stab, out)
    with tile.TileContext(nc) as tc:
        _emit(nc, tc, aps)
    nc.compile()
    return nc


def host_prep(q, k, v, freqs_cis, wq, wk, wv, wo):
    """Build the 8 per-core input maps."""
    q = np.asarray(q, dtype=np.float32)
    k = np.asarray(k, dtype=np.float32)
    v = np.asarray(v, dtype=np.float32)
    fc = np.asarray(freqs_cis, dtype=np.float32)
    wq = np.asarray(wq, dtype=np.float32)
    wk = np.asarray(wk, dtype=np.float32)
    wv = np.asarray(wv, dtype=np.float32)
    wo = np.asarray(wo, dtype=np.float32)

    cos, sin = fc[:, :, 0], fc[:, :, 1]            # (S, 32)
    idx = (np.arange(128) % 64) // 2
    ct = np.ascontiguousarray(cos[:, idx].T)       # (128, S)
    sgn = np.where(np.arange(128) % 2 == 0, -1.0, 1.0).astype(np.float32)
    st = np.ascontiguousarray(sin[:, idx].T * sgn[:, None])

    def b16(a):
        return np.ascontiguousarray(a).astype(NPBF16)

    in_maps = []
    for c in range(NCORES):
        b, g = c // 2, c % 2
        rows = slice(g * E, (g + 1) * E)
        in_maps.append({
            "qT": b16(q[:, b, :].T),
            "kT": b16(k[:, b, :].T),
            "vT": b16(v[:, b, :].T),
            "wqT": b16(wq[rows, :].T),
            "wkT": b16(wk[rows, :].T),
            "wvT": b16(wv[rows, :].T),
            "woT": b16(wo[:, rows].T),
            "ct": b16(ct),
            "st": b16(st),
        })
    return in_maps


def kernel(q, k, v, freqs_cis, wq, wk, wv, wo, trace=False):
    global _PROG, LAST_RESULT
    if _PROG is None:
        _PROG = build_program()
    in_maps = host_prep(q, k, v, freqs_cis, wq, wk, wv, wo)
    res = run_bass_kernel_spmd(_PROG, in_maps, list(range(NCORES)), trace=trace)
    LAST_RESULT = res
    out = np.empty((S, B, D), dtype=np.float32)
    for b in range(B):
        out[:, b, :] = res.results[2 * b]["out"] + res.results[2 * b + 1]["out"]
    return out
